# revision 2
# baseline (speedup 1.0000x reference)
"""Trainium2 Bass kernel v2 for nn_BPFeedForward (per-element-type MLP).

Differences vs v1 (fp32r baseline):
- All matmuls in bf16 (same 1 cyc/col PE rate as fp32r, half the DMA/SBUF).
- Merged-halves PSUM tiles [128, 2*w]: one tanh activation instruction per
  layer-unit instead of two (biases are zero per the problem spec, so one
  fused ACT covers both 128-feature halves). Matmul chunks are kept
  PSUM-bank aligned.
- A fraction of tanh tiles is offloaded from the (bottleneck) Scalar engine
  to the idle Vector engine as a clamped odd polynomial (deg 9 in z,
  density-weighted fit, rms err ~5e-3).
- Lout results for the 4 element types are batched into one PSUM tile at
  partitions 0/32/64/96 (matmul column-quadrant placement), so the
  PSUM->SBUF drain is one DVE copy per span instead of four.

Host side: shard atoms 8 ways, bf16-convert, run SPMD, numpy segment-sum
routing (reference semantics). If any bias is nonzero (never per spec),
fall back to an exact numpy implementation.
"""

import sys

if "/opt/trn_rl_repo" not in sys.path:
    sys.path.insert(0, "/opt/trn_rl_repo")

import numpy as np

N_CORES = 8
E = 4
N_ATOMS = 200000
M_TOTAL = N_ATOMS // E          # 50000 atoms per element type
MPC = M_TOTAL // N_CORES        # 6250 atoms per element per core
D = 128
H = 256
SUPER = 768                     # atoms per span (merged psum = [128, 2*SUPER])
MP = MPC

N_DVE_POLY = 0                  # unit-layers (of 108) offloaded to DVE poly
N_POOL_POLY = 0                 # GPSIMD poly measured ~80us/chain: unusable
# tanh ~= z*P(z^2) on [-C, C]; tight clamp keeps bf16 Horner cancellation
# small and tanh(C)~=1-5e-3 truncation is negligible for N(0,1) inputs
CLAMP = 3.0
COEF = [0.9827085962228131, -0.26684140255152505, 0.055462455572377924,
        -0.006017147296072327, 0.000251749441018762]

_COMPILED = {}

SPANS = []
_pos = 0
while _pos < MP:
    _w = min(SUPER, MP - _pos)
    SPANS.append((_pos, _w))
    _pos += _w


def _half_chunks(ht, w):
    """Bank-aligned chunks of half `ht` inside the merged [128, 2w] tile.
    Returns (global_col, local_col, width)."""
    start, end = ht * w, ht * w + w
    bps = [start]
    b = (start // 512 + 1) * 512
    while b < end:
        bps.append(b)
        b += 512
    bps.append(end)
    return [(a, a - start, b2 - a) for a, b2 in zip(bps, bps[1:])]


def _build_program(reps: int = 1):
    import concourse.mybir as mybir
    import concourse.tile as tile
    from concourse import bacc

    BF = mybir.dt.bfloat16
    F32 = mybir.dt.float32
    Alu = mybir.AluOpType
    Tanh = mybir.ActivationFunctionType.Tanh

    nc = bacc.Bacc(None, target_bir_lowering=False, debug=False)

    xt = nc.dram_tensor("xt", [E, D, MP], BF, kind="ExternalInput")
    w0 = nc.dram_tensor("w0", [128, E, H], BF, kind="ExternalInput")
    w1 = nc.dram_tensor("w1", [128, E, 2, H], BF, kind="ExternalInput")
    w2 = nc.dram_tensor("w2", [128, E, 2, H], BF, kind="ExternalInput")
    wo = nc.dram_tensor("wo", [128, E, 2], BF, kind="ExternalInput")
    out = nc.dram_tensor("out", [E, MP], F32, kind="ExternalOutput")

    units = [(e, c0, w) for (c0, w) in SPANS for e in range(E)]
    n_units = len(units)

    # Spread poly unit-layers evenly over all (unit, layer) pairs,
    # interleaving DVE and Pool chains proportionally.
    n_ul = n_units * 3
    poly_eng = {}
    n_poly = N_DVE_POLY + N_POOL_POLY
    if n_poly > 0:
        step = n_ul / n_poly
        picks = sorted({int(i * step) for i in range(n_poly)})
        kinds = []
        nd = np_ = 0
        for j in range(len(picks)):
            # keep running ratio close to N_DVE_POLY : N_POOL_POLY
            if nd * N_POOL_POLY <= np_ * N_DVE_POLY:
                kinds.append("dve"); nd += 1
            else:
                kinds.append("pool"); np_ += 1
        for idx, k in zip(picks, kinds):
            poly_eng[idx] = k

    with tile.TileContext(nc) as tc:
        with (
            tc.tile_pool(name="consts", bufs=1) as consts,
            tc.tile_pool(name="xin", bufs=6) as xin,
            tc.tile_pool(name="acts", bufs=14) as actp,
            tc.tile_pool(name="poly", bufs=14) as polp,
            tc.tile_pool(name="osb", bufs=2) as osbp,
            tc.tile_pool(name="psum", bufs=2, space="PSUM") as psp,
            tc.tile_pool(name="psout", bufs=1, space="PSUM") as psop,
        ):
            w0_t = consts.tile([128, E, H], BF)
            nc.sync.dma_start(out=w0_t[:], in_=w0[:])
            w1_t = consts.tile([128, E, 2, H], BF)
            nc.sync.dma_start(out=w1_t[:], in_=w1[:])
            w2_t = consts.tile([128, E, 2, H], BF)
            nc.sync.dma_start(out=w2_t[:], in_=w2[:])
            wo_t = consts.tile([128, E, 2], BF)
            nc.sync.dma_start(out=wo_t[:], in_=wo[:])
            # zeros for the pso-tile init matmul (fills partitions 0..96 so
            # the batched [0:97] drain copy never reads uninitialized PSUM)
            zrow = consts.tile([1, SUPER], BF)
            nc.vector.memset(zrow[:], 0.0)
            zcol = consts.tile([1, 97], BF)
            nc.vector.memset(zcol[:], 0.0)

            all_units = []
            for _rep in range(reps):
                all_units.extend(units)
            n_total = len(all_units)

            xs = [None] * n_total
            a_cur = [None] * n_total
            pso_cur = [None]  # current span-group psum out tile

            pending_tails = []

            def act_or_poly(u, li, ps, w):
                w2c = 2 * w
                a = actp.tile([128, 2 * SUPER], BF, tag="a", name=f"a{li}_{u}")
                idx = (u % n_units) * 3 + li
                kind = poly_eng.get(idx)
                if kind is None:
                    nc.scalar.activation(
                        out=a[:, :w2c], in_=ps[:, :w2c], func=Tanh)
                    return a
                # Clamp now (frees the PSUM slot quickly); defer the rest of
                # the polynomial to the next pipeline step so later clamps
                # and copies are not stuck behind it in the DVE FIFO.
                zc = polp.tile([128, 2 * SUPER], BF, tag="zc")
                nc.vector.tensor_scalar(
                    out=zc[:, :w2c], in0=ps[:, :w2c],
                    scalar1=float(CLAMP), scalar2=float(-CLAMP),
                    op0=Alu.min, op1=Alu.max)

                def tail(zc=zc, a=a, w2c=w2c):
                    # TT/TS Horner (tensor_tensor 2x + tensor_scalar 4x modes)
                    eng = nc.vector
                    t = polp.tile([128, 2 * SUPER], BF, tag="t")
                    eng.tensor_tensor(
                        out=t[:, :w2c], in0=zc[:, :w2c], in1=zc[:, :w2c],
                        op=Alu.mult)
                    p = polp.tile([128, 2 * SUPER], BF, tag="p")
                    eng.tensor_scalar(
                        out=p[:, :w2c], in0=t[:, :w2c],
                        scalar1=float(COEF[4]), scalar2=float(COEF[3]),
                        op0=Alu.mult, op1=Alu.add)
                    for c in (COEF[2], COEF[1], COEF[0]):
                        p2 = polp.tile([128, 2 * SUPER], BF, tag="p")
                        eng.tensor_tensor(
                            out=p2[:, :w2c], in0=p[:, :w2c], in1=t[:, :w2c],
                            op=Alu.mult)
                        p3 = polp.tile([128, 2 * SUPER], BF, tag="p")
                        eng.tensor_scalar(
                            out=p3[:, :w2c], in0=p2[:, :w2c],
                            scalar1=float(c), scalar2=None, op0=Alu.add)
                        p = p3
                    eng.tensor_tensor(
                        out=a[:, :w2c], in0=p[:, :w2c], in1=zc[:, :w2c],
                        op=Alu.mult)

                pending_tails.append(tail)
                return a

            def s0_load(u):
                e, c0, w = all_units[u]
                x = xin.tile([128, SUPER], BF, tag="x", name=f"x{u}")
                nc.sync.dma_start(out=x[:, :w], in_=xt[e, :, c0:c0 + w])
                xs[u] = x

            def s1_layer0(u):
                e, c0, w = all_units[u]
                ps = psp.tile([128, 2 * SUPER], F32, tag="ps", name=f"ps0_{u}")
                for ht in range(2):
                    for gc, lc, cw in _half_chunks(ht, w):
                        nc.tensor.matmul(
                            ps[:, gc:gc + cw],
                            w0_t[:, e, ht * 128:(ht + 1) * 128],
                            xs[u][:, lc:lc + cw],
                        )
                xs[u] = None
                a_cur[u] = act_or_poly(u, 0, ps, w)

            def mid_layer(u, w_t, li):
                e, c0, w = all_units[u]
                prev = a_cur[u]
                ps = psp.tile([128, 2 * SUPER], F32, tag="ps",
                              name=f"ps{li}_{u}")
                for ht in range(2):
                    for gc, lc, cw in _half_chunks(ht, w):
                        for kt in range(2):
                            nc.tensor.matmul(
                                ps[:, gc:gc + cw],
                                w_t[:, e, kt, ht * 128:(ht + 1) * 128],
                                prev[:, kt * w + lc: kt * w + lc + cw],
                                start=(kt == 0),
                                stop=(kt == 1),
                            )
                a_cur[u] = act_or_poly(u, li, ps, w)

            def s2_layer1(u):
                mid_layer(u, w1_t, 1)

            def s3_layer2(u):
                mid_layer(u, w2_t, 2)

            def s4_out(u):
                e, c0, w = all_units[u]
                prev = a_cur[u]
                if e == 0:
                    pso_cur[0] = psop.tile([128, SUPER], F32, tag="pso",
                                           name=f"pso_{u}")
                    for gc, lc, cw in _half_chunks(0, w):
                        nc.tensor.matmul(
                            pso_cur[0][0:97, gc:gc + cw], zcol[:1, :97],
                            zrow[:1, lc:lc + cw])
                pso = pso_cur[0]
                for gc, lc, cw in _half_chunks(0, w):
                    for kt in range(2):
                        nc.tensor.matmul(
                            pso[32 * e:32 * e + 1, gc:gc + cw],
                            wo_t[:, e, kt:kt + 1],
                            prev[:, kt * w + lc: kt * w + lc + cw],
                            start=(kt == 0),
                            stop=(kt == 1),
                            tile_position=(0, 32 * e),
                        )
                a_cur[u] = None
                if e == E - 1:
                    o = osbp.tile([128, SUPER], F32, tag="osb")
                    nc.vector.tensor_copy(out=o[0:97, :w], in_=pso[0:97, :w])
                    nc.sync.dma_start(out=out[0:E, c0:c0 + w],
                                      in_=o[0:97:32, :w])

            SKEW = 3
            s0_load(0)
            for t in range(n_total + 3 * SKEW):
                flush = list(pending_tails)
                pending_tails.clear()
                if 0 <= t - 3 * SKEW < n_total:
                    s4_out(t - 3 * SKEW)
                if t + 1 < n_total:
                    s0_load(t + 1)
                if t < n_total:
                    s1_layer0(t)
                if 0 <= t - SKEW < n_total:
                    s2_layer1(t - SKEW)
                if 0 <= t - 2 * SKEW < n_total:
                    s3_layer2(t - 2 * SKEW)
                for tail in flush:
                    tail()
            for tail in pending_tails:
                tail()
            pending_tails.clear()

    nc.compile()
    return nc


def _get_compiled():
    if "v2" not in _COMPILED:
        _COMPILED["v2"] = _build_program(reps=1)
    return _COMPILED["v2"]


def _prep_core_inputs(fps, W0, b0, W1, b1, W2, b2, Wout):
    import ml_dtypes
    BF = ml_dtypes.bfloat16

    def cvt(a):
        return np.ascontiguousarray(a).astype(BF)

    w0_dev = cvt(np.transpose(W0, (1, 0, 2)))
    w1_dev = cvt(W1.reshape(E, 2, 128, H).transpose(2, 0, 1, 3))
    w2_dev = cvt(W2.reshape(E, 2, 128, H).transpose(2, 0, 1, 3))
    wo_dev = cvt(Wout.reshape(E, 2, 128).transpose(2, 0, 1))

    in_maps = []
    for c in range(N_CORES):
        xtc = np.zeros((E, D, MP), BF)
        for e in range(E):
            xtc[e] = fps[e][c * MPC:(c + 1) * MPC].T.astype(BF)
        in_maps.append({
            "xt": xtc,
            "w0": w0_dev, "w1": w1_dev, "w2": w2_dev, "wo": wo_dev,
        })
    return in_maps


def _route_outputs(flat_per_elem, elems, n_atoms):
    out = np.zeros((n_atoms,), np.float32)
    for e in range(E):
        idx_e = np.nonzero(elems == e)[0]
        if idx_e.shape[0] >= M_TOTAL:
            idx_e = idx_e[:M_TOTAL]
        else:
            idx_e = np.concatenate(
                [idx_e, np.zeros(M_TOTAL - idx_e.shape[0], idx_e.dtype)])
        np.add.at(out, idx_e, flat_per_elem[e])
    return out


def _numpy_reference(fps, W0, b0, W1, b1, W2, b2, Wout, elems, n_atoms):
    """Exact fallback (used only if biases are nonzero, which the problem
    spec precludes)."""
    flat = np.empty((E, M_TOTAL), np.float32)
    for e in range(E):
        h = np.tanh(fps[e] @ W0[e] + b0[e])
        h = np.tanh(h @ W1[e] + b1[e])
        h = np.tanh(h @ W2[e] + b2[e])
        flat[e] = (h @ Wout[e])[:, 0]
    return _route_outputs(flat, elems, n_atoms)


def kernel(fps_0, fps_1, fps_2, fps_3, W0, b0, W1, b1, W2, b2, Wout,
           elems, ind_1):
    from concourse.bass_utils import run_bass_kernel_spmd

    f32 = np.float32
    fps = [np.asarray(f, dtype=f32) for f in (fps_0, fps_1, fps_2, fps_3)]
    W0 = np.asarray(W0, dtype=f32)
    W1 = np.asarray(W1, dtype=f32)
    W2 = np.asarray(W2, dtype=f32)
    Wout = np.asarray(Wout, dtype=f32)
    b0 = np.asarray(b0, dtype=f32)
    b1 = np.asarray(b1, dtype=f32)
    b2 = np.asarray(b2, dtype=f32)
    elems = np.asarray(elems)
    n_atoms = np.asarray(ind_1).shape[0]

    if any(np.any(b) for b in (b0, b1, b2)):
        out = _numpy_reference(fps, W0, b0, W1, b1, W2, b2, Wout, elems,
                               n_atoms)
        return out.reshape(n_atoms, 1).astype(f32)

    nc = _get_compiled()
    in_maps = _prep_core_inputs(fps, W0, b0, W1, b1, W2, b2, Wout)
    res = run_bass_kernel_spmd(nc, in_maps, core_ids=list(range(N_CORES)))

    flat = np.empty((E, M_TOTAL), f32)
    for c in range(N_CORES):
        o = res.results[c]["out"]          # [E, MP]
        flat[:, c * MPC:(c + 1) * MPC] = o[:, :MPC]

    out = _route_outputs(flat, elems, n_atoms)
    return out.reshape(n_atoms, 1).astype(f32)
